# revision 21
# baseline (speedup 1.0000x reference)
"""Trainium2 Bass kernel for nn_Dsa_Decoder.

Math note (why this kernel is small): in the reference,
``beta = log_softmax(score, axis=-1)`` is taken over a singleton axis, so
``beta`` is exactly 0 and the context vector ``ctx2 = einsum(beta, enc_h)``
is exactly zero at every step. Each step's LSTM input is therefore
``x = d_t * dense_w[0,0] + dense_b`` (the ctx part of the dense layer
contributes exactly +0.0), and the LSTM always restarts from (h0, c0), so
step outputs are independent across time: the scan's final carry is just
the last step's ``h_s`` plus a zero context. The full module collapses to
one LSTM cell evaluated at ``d = t[:, -1]``:

    gates = [h0 | x | 1] @ [w_hh.T ; w_ih.T ; (b_ih+b_hh)]      (B, 4H)
    c2 = sigmoid(f) * c0 + sigmoid(i) * tanh(g)
    h2 = sigmoid(o) * tanh(c2)
    out = concat([h2, zeros], -1)                               (B, 1, 2H)

Sharding: pure data parallel — batch 512 split across 8 cores (64 rows
each); the tiny weights are replicated. enc_h and the attention weights
never reach the device (they only feed the exactly-zero branch).

Implementation: raw Bass (no TileContext) with hand-placed semaphores, to
avoid Tile's end-of-kernel drain + double all-engine barrier. All device
inputs are packed into ONE dram tensor (two DMAs on sync's HWDGE queue:
the matmul operands first — PE is gated only on those — then c0, which
DVE needs much later). Cross-engine completion signaling uses
drain + sem_inc (chunk-count independent); semaphores are cleared by
their last waiter so the NEFF is safely re-executable without any end
barrier, and the framework's init barrier + unused const memsets are
stripped from the program head. Measured (gauge "useful" exec time,
neuron-profile on core 0 of 8): ~15.1 us, of which ~7 us is the fixed
NEFF teardown and ~2.4 us the runtime-pinned span floor.

Per-core device program:
  sync:   dma(aT|w); dma(c0); wait v>=4; dma(h2 out); wait d_out; clears
  PE:     wait d_in; matmul gates(64x256) = [aT].T @ [w] (K=66, fp32);
          drain; inc p
  gpsimd: memset scratch; drain; inc g     (feeds the table-load dummy)
  ACT:    [ACT_TABLE_LOAD lands here]; wait g; dummy sigmoid(scratch);
          wait p; tanh(g-cols); sigmoid(i|f|o cols); drain; inc a;
          wait v>=3; tanh(c2); drain; inc a
  DVE:    wait a>=1; i*tanh_g; wait d_c; f*c0; drain; add -> c2; drain;
          inc v+=3; wait a>=2; o*tanh_c2; drain; inc v
Gate columns are pre-permuted to [i | f | o | g] so the three sigmoids are
a single ACT instruction.
"""

import numpy as np

import concourse.bass as bass
import concourse.bacc as bacc
import concourse.mybir as mybir
from concourse import bass_utils

B, T, H = 512, 64, 64
N_CORES = 8
BP = B // N_CORES          # 64 batch rows per core
K = H + 2                  # contraction dim: 64 h + 1 x + 1 bias row
G4 = 4 * H                 # 256 gate columns
PACK_W = H + G4 + H        # 384: [aT | w | c0]

_NC_CACHE = None


def _build_nc(sem_clears=True, detect_races=False, out_wait=True):
    """Build + compile the per-core Bass program (cached across calls).

    sem_clears=True restores all semaphores to 0 at the end of the
    program so the NEFF is safely re-executable. The clears are placed on
    each semaphore's final observer (safe: executions serialize at NEFF
    boundaries), which the CoreSim race checker can't prove — so race
    validation (sim_check.py) uses a sem_clears=False build and numerics
    use this one with the checker off.
    """
    global _NC_CACHE
    if _NC_CACHE is not None and sem_clears and not detect_races and out_wait:
        return _NC_CACHE

    nc = bacc.Bacc("TRN2", target_bir_lowering=False, debug=False,
                   num_devices=N_CORES, detect_race_conditions=detect_races)
    f32 = mybir.dt.float32
    AF = mybir.ActivationFunctionType
    packed_d = nc.dram_tensor("packed", (K, PACK_W), f32, kind="ExternalInput")
    h2_d = nc.dram_tensor("h2", (BP, H), f32, kind="ExternalOutput")

    with (
        nc.sbuf_tensor("sb", [K, PACK_W], f32) as sb,
        nc.sbuf_tensor("sig", [BP, 3 * H], f32) as sig,
        nc.sbuf_tensor("tg", [BP, H], f32) as tg,
        nc.sbuf_tensor("t1", [BP, H], f32) as t1,
        nc.sbuf_tensor("t2", [BP, H], f32) as t2,
        nc.sbuf_tensor("c2", [BP, H], f32) as c2,
        nc.sbuf_tensor("tc2", [BP, H], f32) as tc2,
        nc.sbuf_tensor("h2_sb", [BP, H], f32) as h2,
        nc.sbuf_tensor("scratch", [BP, 1], f32) as scratch,
        nc.psum_tensor("gates", [BP, G4], f32) as gates,
        nc.semaphore("d_in") as d_in,
        nc.semaphore("d_c") as d_c,
        nc.semaphore("d_out") as d_out,
        nc.semaphore("p") as p,
        nc.semaphore("a") as a,
        nc.semaphore("v") as v,
        nc.semaphore("g") as g,
    ):
        sy, pe, act, dve = nc.sync, nc.tensor, nc.scalar, nc.vector
        MM_W = H + G4          # 320: the [aT | w] region the matmul needs

        # sync: input DMAs (matmul part first — PE is gated only on it;
        # c0 follows on the same queue and is only needed much later by
        # DVE) + the output DMA. Sem clears are placed after a later
        # instruction so the pending wait_ge nop-fuses onto a non-clear
        # instruction (the race checker requires updates to be consumed
        # by a wait that precedes the clear).
        sy.dma_start(sb[:, 0:MM_W], packed_d[:, 0:MM_W]).then_inc(d_in, 16)
        sy.dma_start(sb[0:BP, MM_W:PACK_W],
                     packed_d[0:BP, MM_W:PACK_W]).then_inc(d_c, 16)
        sy.wait_ge(v, 4)
        if out_wait:
            sy.dma_start(h2_d[:], h2[:]).then_inc(d_out, 16)
            if sem_clears:
                sy.sem_clear(v)
            sy.wait_ge(d_out, 16)
            if sem_clears:
                sy.sem_clear(d_out)
        else:
            # Sem update attached (framework requires one) but nobody
            # waits: completion is covered by the NEFF teardown, which
            # runs ~7us of drains/barriers after this point while the
            # 16KB transfer needs <1us. d_out accumulates across
            # executions, which is harmless since nothing reads it.
            sy.dma_start(h2_d[:], h2[:]).then_inc(d_out, 16)
            if sem_clears:
                sy.sem_clear(v)

        # PE: single matmul, contraction over K=66. Instructions may lower
        # to several ISA chunks, each of which re-fires a then_inc — so all
        # compute-completion signaling below uses explicit drain + sem_inc,
        # which is chunk-count independent.
        pe.wait_ge(d_in, 16)
        pe.matmul(gates[:], sb[:, 0:H], sb[:, H:H + G4], start=True, stop=True)
        pe.drain()
        if sem_clears:
            pe.sem_clear(d_in)
        pe.sem_inc(p, 1)

        # GpSimd: initialize the dummy-activation scratch (the framework
        # const memsets are stripped below, and the simulator refuses
        # uninitialized reads).
        gp = nc.gpsimd
        gp.memset(scratch[:], 0.0)
        gp.drain()
        gp.sem_inc(g, 1)

        # ACT: dummy activation so Bacc's table-load pass puts the single
        # ACT_TABLE_LOAD at program start — overlapping the DMA + matmul —
        # instead of behind the wait on the matmul.
        act.wait_ge(g, 1)
        act.activation(scratch[:], scratch[:], AF.Sigmoid)
        act.wait_ge(p, 1)
        act.activation(tg[:], gates[:, 3 * H:G4], AF.Tanh)
        act.activation(sig[:], gates[:, 0:3 * H], AF.Sigmoid)
        act.drain()
        if sem_clears:
            act.sem_clear(p)
            act.sem_clear(g)
        act.sem_inc(a, 1)
        act.wait_ge(v, 3)
        act.activation(tc2[:], c2[:], AF.Tanh)
        act.drain()
        act.sem_inc(a, 1)

        # DVE: gate combine
        dve.wait_ge(a, 1)
        dve.tensor_mul(t2[:], sig[:, 0:H], tg[:])                      # i*tanh(g)
        dve.wait_ge(d_c, 16)
        dve.tensor_mul(t1[:], sig[:, H:2 * H],
                       sb[0:BP, H + G4:PACK_W])                        # f*c0
        dve.drain()                # DVE is pipelined: RAW on t1/t2 needs sync
        if sem_clears:
            dve.sem_clear(d_c)
        dve.tensor_add(c2[:], t1[:], t2[:])
        dve.drain()
        dve.sem_inc(v, 3)
        dve.wait_ge(a, 2)
        dve.tensor_mul(h2[:], sig[:, 2 * H:3 * H], tc2[:])
        dve.drain()
        if sem_clears:
            dve.sem_clear(a)
        dve.sem_inc(v, 1)

    # Strip the framework preamble: three unused const-tensor memsets and
    # the initial all-engine barrier (its gather/release sems end
    # balanced, so removal is re-execution safe; nothing else orders
    # against it). const-float32-0.0 stays — activations read it as the
    # default bias — and is ordered before every ACT instruction via the
    # gpsimd scratch memset -> g semaphore -> ACT program order.
    # Saves ~0.6-0.9us of dead time before the first input DMA.
    blk = nc.main_func.blocks[0]
    for inst in [i for i in blk.instructions
                 if ('const-' in i.concise() and 'Memset' in i.concise()
                     and 'const-float32-0.0' not in i.concise())
                 or 'barrier_Pool_Activation_PE_DVE_SP' in i.concise()]:
        blk.instructions.remove(inst)

    nc.compile()
    if sem_clears and not detect_races and out_wait:
        _NC_CACHE = nc
    return nc


def _pack_inputs(t, h0, c0, dense_w, dense_b, w_ih, w_hh, b_ih, b_hh):
    """Host-side shard + layout packing (tiny: O(B*H + H^2) floats)."""
    d = t[:, -1]                                    # (B,) last time step
    x = d * dense_w[0, 0] + dense_b[0]              # (B,) dense layer on [d, 0ctx]

    # Gate columns permuted to [i | f | o | g].
    perm = np.concatenate([np.arange(0, H), np.arange(H, 2 * H),
                           np.arange(3 * H, 4 * H), np.arange(2 * H, 3 * H)])
    w = np.empty((K, G4), np.float32)
    w[:H] = w_hh.T[:, perm]
    w[H] = w_ih[perm, 0]
    w[H + 1] = (b_ih + b_hh)[perm]

    h = h0[0]                                       # (B, H)
    c = c0[0]                                       # (B, H)
    in_maps = []
    for core in range(N_CORES):
        r = slice(core * BP, (core + 1) * BP)
        packed = np.zeros((K, PACK_W), np.float32)
        packed[:H, 0:H] = h[r].T                    # aT rows 0:64
        packed[H, 0:H] = x[r]                       # x row
        packed[H + 1, 0:H] = 1.0                    # ones row
        packed[:, H:H + G4] = w
        packed[0:BP, H + G4:PACK_W] = c[r]          # c0 block
        in_maps.append({"packed": packed})
    return in_maps


def kernel(t, enc_h, h0, c0, dense_w, dense_b, w_ih, w_hh, b_ih, b_hh,
           w1_w, w1_b, w2_w, w2_b, v_w, v_b, **_unused):
    t = np.asarray(t, np.float32)
    h0 = np.asarray(h0, np.float32)
    c0 = np.asarray(c0, np.float32)
    dense_w = np.asarray(dense_w, np.float32)
    dense_b = np.asarray(dense_b, np.float32)
    w_ih = np.asarray(w_ih, np.float32)
    w_hh = np.asarray(w_hh, np.float32)
    b_ih = np.asarray(b_ih, np.float32)
    b_hh = np.asarray(b_hh, np.float32)

    nc = _build_nc()
    in_maps = _pack_inputs(t, h0, c0, dense_w, dense_b, w_ih, w_hh, b_ih, b_hh)
    res = bass_utils.run_bass_kernel_spmd(nc, in_maps, core_ids=list(range(N_CORES)))

    h2 = np.concatenate([res.results[c]["h2"] for c in range(N_CORES)], axis=0)
    out = np.zeros((B, 1, 2 * H), np.float32)
    out[:, 0, :H] = h2
    return out


# revision 22
# speedup vs baseline: 1.5506x; 1.5506x over previous
"""Trainium2 Bass kernel for nn_Dsa_Decoder.

Math note (why this kernel is small): in the reference,
``beta = log_softmax(score, axis=-1)`` is taken over a singleton axis, so
``beta`` is exactly 0 and the context vector ``ctx2 = einsum(beta, enc_h)``
is exactly zero at every step. Each step's LSTM input is therefore
``x = d_t * dense_w[0,0] + dense_b`` (the ctx part of the dense layer
contributes exactly +0.0), and the LSTM always restarts from (h0, c0), so
step outputs are independent across time: the scan's final carry is just
the last step's ``h_s`` plus a zero context. The full module collapses to
one LSTM cell evaluated at ``d = t[:, -1]``:

    gates = [h0 | x | 1] @ [w_hh.T ; w_ih.T ; (b_ih+b_hh)]      (B, 4H)
    c2 = sigmoid(f) * c0 + sigmoid(i) * tanh(g)
    h2 = sigmoid(o) * tanh(c2)
    out = concat([h2, zeros], -1)                               (B, 1, 2H)

Sharding: pure data parallel — batch 512 split across 8 cores (64 rows
each); the tiny weights are replicated. enc_h and the attention weights
never reach the device (they only feed the exactly-zero branch).

Implementation: raw Bass (no TileContext) with hand-placed semaphores, to
avoid Tile's end-of-kernel drain + double all-engine barrier. All device
inputs are packed into ONE dram tensor (two DMAs on sync's HWDGE queue:
the matmul operands first — PE is gated only on those — then c0, which
DVE needs much later). Cross-engine completion signaling uses
drain + sem_inc (chunk-count independent); semaphores are cleared by
their last waiter so the NEFF is safely re-executable without any end
barrier, and the framework's init barrier + unused const memsets are
stripped from the program head. Measured (gauge "useful" exec time,
neuron-profile on core 0 of 8): ~15.1 us, of which ~7 us is the fixed
NEFF teardown and ~2.4 us the runtime-pinned span floor.

Per-core device program:
  sync:   dma(aT|w); dma(c0); wait v>=4; dma(h2 out); wait d_out; clears
  PE:     wait d_in; matmul gates(64x256) = [aT].T @ [w] (K=66, fp32);
          drain; inc p
  gpsimd: memset scratch; drain; inc g     (feeds the table-load dummy)
  ACT:    [ACT_TABLE_LOAD lands here]; wait g; dummy sigmoid(scratch);
          wait p; tanh(g-cols); sigmoid(i|f|o cols); drain; inc a;
          wait v>=3; tanh(c2); drain; inc a
  DVE:    wait a>=1; i*tanh_g; wait d_c; f*c0; drain; add -> c2; drain;
          inc v+=3; wait a>=2; o*tanh_c2; drain; inc v
Gate columns are pre-permuted to [i | f | o | g] so the three sigmoids are
a single ACT instruction.
"""

import numpy as np

import concourse.bass as bass
import concourse.bacc as bacc
import concourse.mybir as mybir
from concourse import bass_utils

B, T, H = 512, 64, 64
N_CORES = 8
BP = B // N_CORES          # 64 batch rows per core
K = H + 2                  # contraction dim: 64 h + 1 x + 1 bias row
G4 = 4 * H                 # 256 gate columns
PACK_W = H + G4 + H        # 384: [aT | w | c0]

_NC_CACHE = None


def _build_nc(sem_clears=True, detect_races=False, out_wait=True):
    """Build + compile the per-core Bass program (cached across calls).

    sem_clears=True restores all semaphores to 0 at the end of the
    program so the NEFF is safely re-executable. The clears are placed on
    each semaphore's final observer (safe: executions serialize at NEFF
    boundaries), which the CoreSim race checker can't prove — so race
    validation (sim_check.py) uses a sem_clears=False build and numerics
    use this one with the checker off.
    """
    global _NC_CACHE
    if _NC_CACHE is not None and sem_clears and not detect_races and out_wait:
        return _NC_CACHE

    nc = bacc.Bacc("TRN2", target_bir_lowering=False, debug=False,
                   num_devices=N_CORES, detect_race_conditions=detect_races)
    f32 = mybir.dt.float32
    AF = mybir.ActivationFunctionType
    packed_d = nc.dram_tensor("packed", (K, PACK_W), f32, kind="ExternalInput")
    h2_d = nc.dram_tensor("h2", (BP, H), f32, kind="ExternalOutput")

    with (
        nc.sbuf_tensor("sb", [K, PACK_W], f32) as sb,
        nc.sbuf_tensor("sig", [BP, 3 * H], f32) as sig,
        nc.sbuf_tensor("tg", [BP, H], f32) as tg,
        nc.sbuf_tensor("t1", [BP, H], f32) as t1,
        nc.sbuf_tensor("t2", [BP, H], f32) as t2,
        nc.sbuf_tensor("c2", [BP, H], f32) as c2,
        nc.sbuf_tensor("tc2", [BP, H], f32) as tc2,
        nc.sbuf_tensor("h2_sb", [BP, H], f32) as h2,
        nc.sbuf_tensor("scratch", [BP, 1], f32) as scratch,
        nc.psum_tensor("gates", [BP, G4], f32) as gates,
        nc.semaphore("d_in") as d_in,
        nc.semaphore("d_c") as d_c,
        nc.semaphore("d_out") as d_out,
        nc.semaphore("p") as p,
        nc.semaphore("a") as a,
        nc.semaphore("v") as v,
        nc.semaphore("g") as g,
    ):
        sy, pe, act, dve = nc.sync, nc.tensor, nc.scalar, nc.vector
        MM_W = H + G4          # 320: the [aT | w] region the matmul needs

        # sync: input DMAs (matmul part first — PE is gated only on it;
        # c0 follows on the same queue and is only needed much later by
        # DVE) + the output DMA. Sem clears are placed after a later
        # instruction so the pending wait_ge nop-fuses onto a non-clear
        # instruction (the race checker requires updates to be consumed
        # by a wait that precedes the clear).
        sy.dma_start(sb[:, 0:MM_W], packed_d[:, 0:MM_W]).then_inc(d_in, 16)
        sy.dma_start(sb[0:BP, MM_W:PACK_W],
                     packed_d[0:BP, MM_W:PACK_W]).then_inc(d_c, 16)
        sy.wait_ge(v, 4)
        if out_wait:
            sy.dma_start(h2_d[:], h2[:]).then_inc(d_out, 16)
            if sem_clears:
                sy.sem_clear(v)
            sy.wait_ge(d_out, 16)
            if sem_clears:
                sy.sem_clear(d_out)
        else:
            # Sem update attached (framework requires one) but nobody
            # waits: completion is covered by the NEFF teardown, which
            # runs ~7us of drains/barriers after this point while the
            # 16KB transfer needs <1us. d_out accumulates across
            # executions, which is harmless since nothing reads it.
            sy.dma_start(h2_d[:], h2[:]).then_inc(d_out, 16)
            if sem_clears:
                sy.sem_clear(v)

        # PE: single matmul, contraction over K=66. Instructions may lower
        # to several ISA chunks, each of which re-fires a then_inc — so all
        # compute-completion signaling below uses explicit drain + sem_inc,
        # which is chunk-count independent.
        pe.wait_ge(d_in, 16)
        pe.matmul(gates[:], sb[:, 0:H], sb[:, H:H + G4], start=True, stop=True)
        pe.drain()
        if sem_clears:
            pe.sem_clear(d_in)
        pe.sem_inc(p, 1)

        # GpSimd: initialize the dummy-activation scratch (the framework
        # const memsets are stripped below, and the simulator refuses
        # uninitialized reads).
        gp = nc.gpsimd
        gp.memset(scratch[:], 0.0)
        gp.drain()
        gp.sem_inc(g, 1)

        # ACT: dummy activation so Bacc's table-load pass puts the single
        # ACT_TABLE_LOAD at program start — overlapping the DMA + matmul —
        # instead of behind the wait on the matmul.
        act.wait_ge(g, 1)
        act.activation(scratch[:], scratch[:], AF.Sigmoid)
        act.wait_ge(p, 1)
        act.activation(tg[:], gates[:, 3 * H:G4], AF.Tanh)
        act.activation(sig[:], gates[:, 0:3 * H], AF.Sigmoid)
        act.drain()
        if sem_clears:
            act.sem_clear(p)
            act.sem_clear(g)
        act.sem_inc(a, 1)
        act.wait_ge(v, 3)
        act.activation(tc2[:], c2[:], AF.Tanh)
        act.drain()
        act.sem_inc(a, 1)

        # DVE: gate combine
        dve.wait_ge(a, 1)
        dve.tensor_mul(t2[:], sig[:, 0:H], tg[:])                      # i*tanh(g)
        dve.wait_ge(d_c, 16)
        dve.tensor_mul(t1[:], sig[:, H:2 * H],
                       sb[0:BP, H + G4:PACK_W])                        # f*c0
        dve.drain()                # DVE is pipelined: RAW on t1/t2 needs sync
        if sem_clears:
            dve.sem_clear(d_c)
        dve.tensor_add(c2[:], t1[:], t2[:])
        dve.drain()
        dve.sem_inc(v, 3)
        dve.wait_ge(a, 2)
        dve.tensor_mul(h2[:], sig[:, 2 * H:3 * H], tc2[:])
        dve.drain()
        if sem_clears:
            dve.sem_clear(a)
        dve.sem_inc(v, 1)

    # Strip the framework preamble: three unused const-tensor memsets and
    # the initial all-engine barrier (its gather/release sems end
    # balanced, so removal is re-execution safe; nothing else orders
    # against it). const-float32-0.0 stays — activations read it as the
    # default bias — and is ordered before every ACT instruction via the
    # gpsimd scratch memset -> g semaphore -> ACT program order.
    # Saves ~0.6-0.9us of dead time before the first input DMA.
    blk = nc.main_func.blocks[0]
    for inst in [i for i in blk.instructions
                 if ('const-' in i.concise() and 'Memset' in i.concise()
                     and 'const-float32-0.0' not in i.concise())
                 or 'barrier_Pool_Activation_PE_DVE_SP' in i.concise()]:
        blk.instructions.remove(inst)

    nc.compile()
    if sem_clears and not detect_races and out_wait:
        _NC_CACHE = nc
    return nc


def _pack_inputs(t, h0, c0, dense_w, dense_b, w_ih, w_hh, b_ih, b_hh):
    """Host-side shard + layout packing (tiny: O(B*H + H^2) floats)."""
    d = t[:, -1]                                    # (B,) last time step
    x = d * dense_w[0, 0] + dense_b[0]              # (B,) dense layer on [d, 0ctx]

    # Gate columns permuted to [i | f | o | g].
    perm = np.concatenate([np.arange(0, H), np.arange(H, 2 * H),
                           np.arange(3 * H, 4 * H), np.arange(2 * H, 3 * H)])
    w = np.empty((K, G4), np.float32)
    w[:H] = w_hh.T[:, perm]
    w[H] = w_ih[perm, 0]
    w[H + 1] = (b_ih + b_hh)[perm]

    h = h0[0]                                       # (B, H)
    c = c0[0]                                       # (B, H)
    in_maps = []
    for core in range(N_CORES):
        r = slice(core * BP, (core + 1) * BP)
        packed = np.zeros((K, PACK_W), np.float32)
        packed[:H, 0:H] = h[r].T                    # aT rows 0:64
        packed[H, 0:H] = x[r]                       # x row
        packed[H + 1, 0:H] = 1.0                    # ones row
        packed[:, H:H + G4] = w
        packed[0:BP, H + G4:PACK_W] = c[r]          # c0 block
        in_maps.append({"packed": packed})
    return in_maps


def kernel(t, enc_h, h0, c0, dense_w, dense_b, w_ih, w_hh, b_ih, b_hh,
           w1_w, w1_b, w2_w, w2_b, v_w, v_b, **_unused):
    t = np.asarray(t, np.float32)
    h0 = np.asarray(h0, np.float32)
    c0 = np.asarray(c0, np.float32)
    dense_w = np.asarray(dense_w, np.float32)
    dense_b = np.asarray(dense_b, np.float32)
    w_ih = np.asarray(w_ih, np.float32)
    w_hh = np.asarray(w_hh, np.float32)
    b_ih = np.asarray(b_ih, np.float32)
    b_hh = np.asarray(b_hh, np.float32)

    nc = _build_nc()
    in_maps = _pack_inputs(t, h0, c0, dense_w, dense_b, w_ih, w_hh, b_ih, b_hh)
    res = None
    for attempt in range(5):
        try:
            res = bass_utils.run_bass_kernel_spmd(
                nc, in_maps, core_ids=list(range(N_CORES)))
            break
        except Exception as e:  # noqa: BLE001
            # The terminal-side neuron runtime occasionally reports
            # NRT_EXEC_UNIT_UNRECOVERABLE / UNAVAILABLE transiently and
            # self-heals within a minute or two; retry instead of failing.
            msg = str(e)
            transient = ("UNAVAILABLE" in msg or "unrecoverable" in msg
                         or "UNRECOVERABLE" in msg)
            if attempt == 4 or not transient:
                raise
            import time
            time.sleep(45)

    h2 = np.concatenate([res.results[c]["h2"] for c in range(N_CORES)], axis=0)
    out = np.zeros((B, 1, 2 * H), np.float32)
    out[:, 0, :H] = h2
    return out


# revision 23
# speedup vs baseline: 1.5805x; 1.0193x over previous
"""Trainium2 Bass kernel for nn_Dsa_Decoder.

Math note (why this kernel is small): in the reference,
``beta = log_softmax(score, axis=-1)`` is taken over a singleton axis, so
``beta`` is exactly 0 and the context vector ``ctx2 = einsum(beta, enc_h)``
is exactly zero at every step. Each step's LSTM input is therefore
``x = d_t * dense_w[0,0] + dense_b`` (the ctx part of the dense layer
contributes exactly +0.0), and the LSTM always restarts from (h0, c0), so
step outputs are independent across time: the scan's final carry is just
the last step's ``h_s`` plus a zero context. The full module collapses to
one LSTM cell evaluated at ``d = t[:, -1]``:

    gates = [h0 | x | 1] @ [w_hh.T ; w_ih.T ; (b_ih+b_hh)]      (B, 4H)
    c2 = sigmoid(f) * c0 + sigmoid(i) * tanh(g)
    h2 = sigmoid(o) * tanh(c2)
    out = concat([h2, zeros], -1)                               (B, 1, 2H)

Sharding: pure data parallel — batch 512 split across 8 cores (64 rows
each); the tiny weights are replicated. enc_h and the attention weights
never reach the device (they only feed the exactly-zero branch).

Implementation: raw Bass (no TileContext) with hand-placed semaphores, to
avoid Tile's end-of-kernel drain + double all-engine barrier. All device
inputs are packed into ONE dram tensor (two DMAs on sync's HWDGE queue:
the matmul operands first — PE is gated only on those — then c0, which
DVE needs much later). Cross-engine completion signaling uses
drain + sem_inc (chunk-count independent); semaphores are cleared by
their last waiter so the NEFF is safely re-executable without any end
barrier, and the framework's init barrier + unused const memsets are
stripped from the program head. Measured (gauge "useful" exec time,
neuron-profile on core 0 of 8): ~15.1 us, of which ~7 us is the fixed
NEFF teardown and ~2.4 us the runtime-pinned span floor.

Per-core device program:
  sync:   dma(aT|w); dma(c0); wait v>=4; dma(h2 out); wait d_out; clears
  PE:     wait d_in; matmul gates(64x256) = [aT].T @ [w] (K=66, fp32);
          drain; inc p
  gpsimd: memset scratch; drain; inc g     (feeds the table-load dummy)
  ACT:    [ACT_TABLE_LOAD lands here]; wait g; dummy sigmoid(scratch);
          wait p; tanh(g-cols); sigmoid(i|f|o cols); drain; inc a;
          wait v>=3; tanh(c2); drain; inc a
  DVE:    wait a>=1; i*tanh_g; wait d_c; f*c0; drain; add -> c2; drain;
          inc v+=3; wait a>=2; o*tanh_c2; drain; inc v
Gate columns are pre-permuted to [i | f | o | g] so the three sigmoids are
a single ACT instruction.
"""

import numpy as np

import concourse.bacc as bacc
import concourse.mybir as mybir
from concourse import bass_utils

B, T, H = 512, 64, 64
N_CORES = 8
BP = B // N_CORES          # 64 batch rows per core
K = H + 2                  # contraction dim: 64 h + 1 x + 1 bias row
G4 = 4 * H                 # 256 gate columns
PACK_W = H + G4 + H        # 384: [aT | w | c0]

_NC_CACHE = None


def _build_nc(sem_clears=True, detect_races=False, out_wait=True):
    """Build + compile the per-core Bass program (cached across calls).

    sem_clears=True restores all semaphores to 0 at the end of the
    program so the NEFF is safely re-executable. The clears are placed on
    each semaphore's final observer (safe: executions serialize at NEFF
    boundaries), which the CoreSim race checker can't prove — so race
    validation (sim_check.py) uses a sem_clears=False build and numerics
    use this one with the checker off.
    """
    global _NC_CACHE
    if _NC_CACHE is not None and sem_clears and not detect_races and out_wait:
        return _NC_CACHE

    nc = bacc.Bacc("TRN2", target_bir_lowering=False, debug=False,
                   num_devices=N_CORES, detect_race_conditions=detect_races)
    f32 = mybir.dt.float32
    AF = mybir.ActivationFunctionType
    packed_d = nc.dram_tensor("packed", (K, PACK_W), f32, kind="ExternalInput")
    h2_d = nc.dram_tensor("h2", (BP, H), f32, kind="ExternalOutput")

    with (
        nc.sbuf_tensor("sb", [K, PACK_W], f32) as sb,
        nc.sbuf_tensor("sig", [BP, 3 * H], f32) as sig,
        nc.sbuf_tensor("tg", [BP, H], f32) as tg,
        nc.sbuf_tensor("t1", [BP, H], f32) as t1,
        nc.sbuf_tensor("t2", [BP, H], f32) as t2,
        nc.sbuf_tensor("c2", [BP, H], f32) as c2,
        nc.sbuf_tensor("tc2", [BP, H], f32) as tc2,
        nc.sbuf_tensor("h2_sb", [BP, H], f32) as h2,
        nc.sbuf_tensor("scratch", [BP, 1], f32) as scratch,
        nc.psum_tensor("gates", [BP, G4], f32) as gates,
        nc.semaphore("d_in") as d_in,
        nc.semaphore("d_c") as d_c,
        nc.semaphore("d_out") as d_out,
        nc.semaphore("p") as p,
        nc.semaphore("a") as a,
        nc.semaphore("v") as v,
        nc.semaphore("g") as g,
    ):
        sy, pe, act, dve = nc.sync, nc.tensor, nc.scalar, nc.vector
        MM_W = H + G4          # 320: the [aT | w] region the matmul needs

        # sync: input DMAs (matmul part first — PE is gated only on it;
        # c0 follows on the same queue and is only needed much later by
        # DVE) + the output DMA. Sem clears are placed after a later
        # instruction so the pending wait_ge nop-fuses onto a non-clear
        # instruction (the race checker requires updates to be consumed
        # by a wait that precedes the clear).
        sy.dma_start(sb[:, 0:MM_W], packed_d[:, 0:MM_W]).then_inc(d_in, 16)
        sy.dma_start(sb[0:BP, MM_W:PACK_W],
                     packed_d[0:BP, MM_W:PACK_W]).then_inc(d_c, 16)
        sy.wait_ge(v, 4)
        if out_wait:
            sy.dma_start(h2_d[:], h2[:]).then_inc(d_out, 16)
            if sem_clears:
                sy.sem_clear(v)
            sy.wait_ge(d_out, 16)
            if sem_clears:
                sy.sem_clear(d_out)
        else:
            # Sem update attached (framework requires one) but nobody
            # waits: completion is covered by the NEFF teardown, which
            # runs ~7us of drains/barriers after this point while the
            # 16KB transfer needs <1us. d_out accumulates across
            # executions, which is harmless since nothing reads it.
            sy.dma_start(h2_d[:], h2[:]).then_inc(d_out, 16)
            if sem_clears:
                sy.sem_clear(v)

        # PE: single matmul, contraction over K=66. Instructions may lower
        # to several ISA chunks, each of which re-fires a then_inc — so all
        # compute-completion signaling below uses explicit drain + sem_inc,
        # which is chunk-count independent.
        pe.wait_ge(d_in, 16)
        pe.matmul(gates[:], sb[:, 0:H], sb[:, H:H + G4], start=True, stop=True)
        pe.drain()
        if sem_clears:
            pe.sem_clear(d_in)
        pe.sem_inc(p, 1)

        # GpSimd: initialize the dummy-activation scratch (the framework
        # const memsets are stripped below, and the simulator refuses
        # uninitialized reads).
        gp = nc.gpsimd
        gp.memset(scratch[:], 0.0)
        gp.drain()
        gp.sem_inc(g, 1)

        # ACT: dummy activation so Bacc's table-load pass puts the single
        # ACT_TABLE_LOAD at program start — overlapping the DMA + matmul —
        # instead of behind the wait on the matmul.
        act.wait_ge(g, 1)
        act.activation(scratch[:], scratch[:], AF.Sigmoid)
        act.wait_ge(p, 1)
        act.activation(tg[:], gates[:, 3 * H:G4], AF.Tanh)
        act.activation(sig[:], gates[:, 0:3 * H], AF.Sigmoid)
        act.drain()
        if sem_clears:
            act.sem_clear(p)
            act.sem_clear(g)
        act.sem_inc(a, 1)
        act.wait_ge(v, 3)
        act.activation(tc2[:], c2[:], AF.Tanh)
        act.drain()
        act.sem_inc(a, 1)

        # DVE: gate combine
        dve.wait_ge(a, 1)
        dve.tensor_mul(t2[:], sig[:, 0:H], tg[:])                      # i*tanh(g)
        dve.wait_ge(d_c, 16)
        dve.tensor_mul(t1[:], sig[:, H:2 * H],
                       sb[0:BP, H + G4:PACK_W])                        # f*c0
        dve.drain()                # DVE is pipelined: RAW on t1/t2 needs sync
        if sem_clears:
            dve.sem_clear(d_c)
        dve.tensor_add(c2[:], t1[:], t2[:])
        dve.drain()
        dve.sem_inc(v, 3)
        dve.wait_ge(a, 2)
        dve.tensor_mul(h2[:], sig[:, 2 * H:3 * H], tc2[:])
        dve.drain()
        if sem_clears:
            dve.sem_clear(a)
        dve.sem_inc(v, 1)

    # Strip the framework preamble: three unused const-tensor memsets and
    # the initial all-engine barrier (its gather/release sems end
    # balanced, so removal is re-execution safe; nothing else orders
    # against it). const-float32-0.0 stays — activations read it as the
    # default bias — and is ordered before every ACT instruction via the
    # gpsimd scratch memset -> g semaphore -> ACT program order.
    # Saves ~0.6-0.9us of dead time before the first input DMA.
    blk = nc.main_func.blocks[0]
    for inst in [i for i in blk.instructions
                 if ('const-' in i.concise() and 'Memset' in i.concise()
                     and 'const-float32-0.0' not in i.concise())
                 or 'barrier_Pool_Activation_PE_DVE_SP' in i.concise()]:
        blk.instructions.remove(inst)

    nc.compile()
    if sem_clears and not detect_races and out_wait:
        _NC_CACHE = nc
    return nc


def _pack_inputs(t, h0, c0, dense_w, dense_b, w_ih, w_hh, b_ih, b_hh):
    """Host-side shard + layout packing (tiny: O(B*H + H^2) floats)."""
    d = t[:, -1]                                    # (B,) last time step
    x = d * dense_w[0, 0] + dense_b[0]              # (B,) dense layer on [d, 0ctx]

    # Gate columns permuted to [i | f | o | g].
    perm = np.concatenate([np.arange(0, H), np.arange(H, 2 * H),
                           np.arange(3 * H, 4 * H), np.arange(2 * H, 3 * H)])
    w = np.empty((K, G4), np.float32)
    w[:H] = w_hh.T[:, perm]
    w[H] = w_ih[perm, 0]
    w[H + 1] = (b_ih + b_hh)[perm]

    h = h0[0]                                       # (B, H)
    c = c0[0]                                       # (B, H)
    in_maps = []
    for core in range(N_CORES):
        r = slice(core * BP, (core + 1) * BP)
        packed = np.zeros((K, PACK_W), np.float32)
        packed[:H, 0:H] = h[r].T                    # aT rows 0:64
        packed[H, 0:H] = x[r]                       # x row
        packed[H + 1, 0:H] = 1.0                    # ones row
        packed[:, H:H + G4] = w
        packed[0:BP, H + G4:PACK_W] = c[r]          # c0 block
        in_maps.append({"packed": packed})
    return in_maps


def kernel(t, enc_h, h0, c0, dense_w, dense_b, w_ih, w_hh, b_ih, b_hh,
           w1_w, w1_b, w2_w, w2_b, v_w, v_b, **_unused):
    t = np.asarray(t, np.float32)
    h0 = np.asarray(h0, np.float32)
    c0 = np.asarray(c0, np.float32)
    dense_w = np.asarray(dense_w, np.float32)
    dense_b = np.asarray(dense_b, np.float32)
    w_ih = np.asarray(w_ih, np.float32)
    w_hh = np.asarray(w_hh, np.float32)
    b_ih = np.asarray(b_ih, np.float32)
    b_hh = np.asarray(b_hh, np.float32)

    nc = _build_nc()
    in_maps = _pack_inputs(t, h0, c0, dense_w, dense_b, w_ih, w_hh, b_ih, b_hh)
    res = None
    for attempt in range(5):
        try:
            res = bass_utils.run_bass_kernel_spmd(
                nc, in_maps, core_ids=list(range(N_CORES)))
            break
        except Exception as e:  # noqa: BLE001
            # The terminal-side neuron runtime occasionally reports
            # NRT_EXEC_UNIT_UNRECOVERABLE / UNAVAILABLE transiently and
            # self-heals within a minute or two; retry instead of failing.
            msg = str(e)
            transient = ("UNAVAILABLE" in msg or "unrecoverable" in msg
                         or "UNRECOVERABLE" in msg)
            if attempt == 4 or not transient:
                raise
            import time
            time.sleep(45)

    h2 = np.concatenate([res.results[c]["h2"] for c in range(N_CORES)], axis=0)
    out = np.zeros((B, 1, 2 * H), np.float32)
    out[:, 0, :H] = h2
    return out
